# revision 8
# baseline (speedup 1.0000x reference)
"""Trainium2 Bass kernel for GQA sliding-window causal attention.

Problem: B=2, S=2048, H=32 q-heads, KVH=8 kv-heads, D=128,
sliding window 1024, causal, scale 1/sqrt(128). f32 I/O.

Sharding (8 cores, pure tensor parallel, no collectives): core c gets
kv-head c and its query-head group [4c, 4c+4). Each core computes full
attention for its 4 q-heads over both batch elements; host concatenates
along the head dim.

Per-core algorithm (banded, no online softmax needed since scores are
O(1) and exp never overflows):
  - Q and K live in SBUF transposed: [d=128 partitions, s free].
  - Scores computed transposed, ST[k, q] = (KT_j).T-contracted-with-QT,
    per (512-wide q-block, 128-wide k-tile) over the causal+window band.
  - Band k-tiles are PAIRED into wide PSUM regions (a full 512-col tile
    in bank 0 + a ramp tile in bank 1, or two small ramp tiles packed in
    one bank) so ONE Exp activation covers up to 1024 contiguous columns
    -> 18 activations per (b,h) instead of 36, halving the fixed
    per-instruction overhead on the scalar engine (the critical
    resource).
  - P = exp(SCALE * ST) on ScalarE (scale folded into the activation),
    written as bf16 to SBUF.
  - Causal-diagonal and window-edge tiles are masked AFTER exp by
    multiplying with 0/1 bf16 mask tiles on VectorE (exact zeros).
  - PV: out[q, 0:129] += PT_slice.T @ V'_j where V' has a ones column
    appended -> col 128 accumulates the softmax denominator for free.
  - Normalize: out = psum[:, :128] * reciprocal(psum[:, 128]) on DVE.
  - Software pipelining: QK matmuls for pair p+1 are emitted before the
    exp/mask/PV processing of pair p, so the PE always has independent
    work while ScalarE runs (keeps the PE out of low p-states).
All matmuls bf16 with f32 PSUM accumulation; softmax math in f32.
"""

import numpy as np
import ml_dtypes

B = 2
S = 2048
H = 32
KVH = 8
D = 128
HQ = H // KVH  # q heads per core = 4
W = 1024  # sliding window
SCALE = 0.08838834764831845
N_CORES = 8
BS = B * S  # 4096
NT = S // 128  # 16 k-tiles / q-tiles per sequence
NG = S // 512  # 4 q-blocks per sequence
VW = D + 1  # 129: V width with ones column

_BF16 = ml_dtypes.bfloat16

_CACHE = {}


def _pairs_for_g(g):
    """Static schedule for q-block g: band tiles packed into ST pairs.

    Returns a list of (segs, width) where segs is a list of
    (j, qv, n, col_off) segments laid out contiguously (no garbage
    columns) in a [128, width] PSUM region; a full-512 first segment
    puts the second segment at col 512 (bank boundary).
    """
    q0 = 512 * g
    tiles = []
    for j in range(max(0, 4 * g - 8), 4 * g + 4):
        qv = max(q0, 128 * j)
        qe = min(q0 + 512, 128 * j + 128 + W)
        tiles.append((j, qv, qe - qv))
    fulls = [t for t in tiles if t[2] == 512]
    smalls = sorted([t for t in tiles if t[2] < 512], key=lambda t: -t[2])
    raw = []
    fi = si = 0
    while fi < len(fulls) and si < len(smalls):
        raw.append([fulls[fi], smalls[si]])
        fi += 1
        si += 1
    while fi + 1 < len(fulls):
        raw.append([fulls[fi], fulls[fi + 1]])
        fi += 2
    if fi < len(fulls):
        raw.append([fulls[fi]])
        fi += 1
    while si + 1 < len(smalls):
        a, b = smalls[si], smalls[si + 1]
        assert a[2] + b[2] <= 512
        raw.append([a, b])
        si += 2
    if si < len(smalls):
        raw.append([smalls[si]])
        si += 1
    pairs = []
    for p in raw:
        segs = []
        off = 0
        for (j, qv, n) in p:
            segs.append((j, qv, n, off))
            off = 512 if (off == 0 and n == 512) else off + n
        width = segs[-1][3] + segs[-1][2]
        pairs.append((segs, width))
    return pairs


_PAIRS = [_pairs_for_g(g) for g in range(NG)]


def _build_nc(reps=1, loop_reps=0, opts=None):
    """Build + compile the single-core Bass/Tile program (SPMD across 8).

    reps > 1 unrolls the whole computation inside one NEFF; loop_reps > 0
    instead wraps the body in a hardware For_i loop. Both are used only
    for timing. opts: dict of tuning switches (see _body_once).
    """
    from contextlib import ExitStack

    import concourse.bass as bass
    import concourse.tile as tile
    from concourse import bacc, mybir

    opts = dict(opts or {})
    fp32 = mybir.dt.float32
    bf16 = mybir.dt.bfloat16

    nc = bacc.Bacc("TRN2", target_bir_lowering=False, debug=False,
                   num_devices=N_CORES)

    qt_d = nc.dram_tensor("qt", [HQ, D, BS], bf16, kind="ExternalInput").ap()
    kt_d = nc.dram_tensor("kt", [D, BS], bf16, kind="ExternalInput").ap()
    vv_d = nc.dram_tensor("vv", [B, 128, NT * VW], bf16, kind="ExternalInput").ap()
    mk_d = nc.dram_tensor("mk", [128, 256], bf16, kind="ExternalInput").ap()
    out_d = nc.dram_tensor("out", [HQ, B, S, D], fp32, kind="ExternalOutput").ap()

    with tile.TileContext(nc) as tc, ExitStack() as ctx:
        mask_pool = ctx.enter_context(tc.tile_pool(name="mask", bufs=1))
        kt_pool = ctx.enter_context(tc.tile_pool(name="ktp", bufs=2))
        vv_pool = ctx.enter_context(tc.tile_pool(name="vvp", bufs=2))
        qt_pool = ctx.enter_context(tc.tile_pool(name="qtp", bufs=2))
        pt_pool = ctx.enter_context(tc.tile_pool(name="ptp",
                                                 bufs=opts.get("pt_bufs", 6)))
        osb_pool = ctx.enter_context(tc.tile_pool(name="osb", bufs=6))
        rec_pool = ctx.enter_context(tc.tile_pool(name="rec", bufs=6))
        st_pool = ctx.enter_context(
            tc.tile_pool(name="stp", bufs=opts.get("st_bufs", 2), space="PSUM"))
        acc_pool = ctx.enter_context(
            tc.tile_pool(name="accp", bufs=opts.get("acc_bufs", 4),
                         space="PSUM"))

        masks = mask_pool.tile([128, 256], bf16)
        nc.sync.dma_start(masks[:], mk_d[:])

        pools = (kt_pool, vv_pool, qt_pool, pt_pool, osb_pool, rec_pool,
                 st_pool, acc_pool)
        if loop_reps:
            with tc.For_i(0, loop_reps, 1,
                          hint_engines=tuple(nc.engines)) as _i:
                _body_once(nc, tc, mybir, masks, *pools,
                           qt_d, kt_d, vv_d, out_d, opts)
        else:
            for _rep in range(reps):
                _body_once(nc, tc, mybir, masks, *pools,
                           qt_d, kt_d, vv_d, out_d, opts)

    nc.compile()
    return nc


def _body_once(nc, tc, mybir, masks, kt_pool, vv_pool, qt_pool, pt_pool,
               osb_pool, rec_pool, st_pool, acc_pool, qt_d, kt_d, vv_d,
               out_d, opts=None):
    opts = opts or {}
    split_act = opts.get("split_act", False)  # one ACT per segment (fallback)
    fp32 = mybir.dt.float32
    bf16 = mybir.dt.bfloat16

    # prefetched input tiles, keyed (kind, b[, h]). All input DMAs are
    # chunked (~128 KB) and round-robined over the two HWDGE queues so the
    # first QK can start ~1.5 us in and later prefetches never monopolize
    # a queue. Chunks are 512-col (q/k) / 4-ktile (v) aligned so every
    # compute read lands inside a single chunk (clean subtile deps).
    kv_tiles = {}
    _qrr = [0]

    def _chunked_dma(dst, src, ncols, step):
        for c0 in range(0, ncols, step):
            eng = nc.sync if _qrr[0] % 2 == 0 else nc.scalar
            _qrr[0] += 1
            eng.dma_start(dst[:, c0:c0 + step], src[:, c0:c0 + step])

    def load_kv(b):
        if ("k", b) not in kv_tiles:
            ktt = kt_pool.tile([128, S], bf16, name=f"ktt_{b}")
            vvt = vv_pool.tile([128, NT * VW], bf16, name=f"vvt_{b}")
            _chunked_dma(ktt, kt_d[:, b * S:(b + 1) * S], S, 512)
            _chunked_dma(vvt, vv_d[b], NT * VW, 4 * VW)
            kv_tiles[("k", b)] = ktt
            kv_tiles[("v", b)] = vvt

    def load_q(b, h):
        if ("q", b, h) not in kv_tiles:
            qtt = qt_pool.tile([128, S], bf16, name=f"qtt_{b}_{h}")
            _chunked_dma(qtt, qt_d[h, :, b * S:(b + 1) * S], S, 512)
            kv_tiles[("q", b, h)] = qtt

    load_kv(0)
    load_q(0, 0)

    # flat schedule of all score-pairs across (b, h, g) so the software
    # pipeline (QK of pair w+1 ahead of processing of pair w) crosses
    # q-block/head/batch boundaries without draining
    sched = []
    for b in range(B):
        for h in range(HQ):
            for g in range(NG):
                for p in range(len(_PAIRS[g])):
                    sched.append((b, h, g, p))

    gctx = {}  # (b, h, g) -> {'acc': [...], 'cpv': [...], 'st': [...]}

    def emit_qk(w):
        b, h, g, p = sched[w]
        pairs = _PAIRS[g]
        if p == 0:
            gctx[(b, h, g)] = {
                "cpv": [0, 0, 0, 0],
                "st": [None] * len(pairs),
            }
            if g == 0:
                # prefetch next head's Q (or next batch's K/V/Q)
                if h + 1 < HQ:
                    load_q(b, h + 1)
                elif b + 1 < B:
                    load_kv(b + 1)
                    load_q(b + 1, 0)
        ktt = kv_tiles[("k", b)]
        qtt = kv_tiles[("q", b, h)]
        segs, width = pairs[p]
        st = st_pool.tile([128, 1024], fp32, tag="st",
                          name=f"st_{b}_{h}_{g}_{p}")
        for (j, qv, n, off) in segs:
            nc.tensor.matmul(
                st[:, off:off + n],
                ktt[:, 128 * j:128 * j + 128],
                qtt[:, qv:qv + n],
                start=True, stop=True,
            )
        gctx[(b, h, g)]["st"][p] = st

    def process(w):
        b, h, g, p = sched[w]
        segs, width = _PAIRS[g][p]
        ctx = gctx[(b, h, g)]
        if "acc" not in ctx:
            ctx["acc"] = [acc_pool.tile([128, VW], fp32, tag="acc",
                                        name=f"acc_{b}_{h}_{g}_{s_}")
                          for s_ in range(4)]
        acc = ctx["acc"]
        cpv = ctx["cpv"]
        npv = [min(4 * g + s_, 8) + 1 for s_ in range(4)]
        vvt = kv_tiles[("v", b)]
        st = ctx["st"][p]
        pt = pt_pool.tile([128, 1024], bf16, tag="pt",
                          name=f"pt_{b}_{h}_{g}_{p}")
        if split_act:
            for (j, qv, n, off) in segs:
                nc.scalar.activation(
                    pt[:, off:off + n], st[:, off:off + n],
                    mybir.ActivationFunctionType.Exp, scale=SCALE)
        else:
            nc.scalar.activation(
                pt[:, 0:width], st[:, 0:width],
                mybir.ActivationFunctionType.Exp, scale=SCALE)
        for (j, qv, n, off) in segs:
            if j >= 4 * g:
                # causal diagonal tile: first 128 cols of seg
                nc.vector.tensor_mul(
                    pt[:, off:off + 128], pt[:, off:off + 128],
                    masks[:, 0:128])
            if qv + n == 128 * j + 128 + W:
                # window edge tile: last 128 cols of seg
                nc.vector.tensor_mul(
                    pt[:, off + n - 128:off + n],
                    pt[:, off + n - 128:off + n],
                    masks[:, 128:256])
        for (j, qv, n, off) in segs:
            for i in range(max(4 * g, j), min(4 * g + 3, j + 8) + 1):
                s_ = i - 4 * g
                po = off + 128 * i - qv
                nc.tensor.matmul(
                    acc[s_][:, :],
                    pt[:, po:po + 128],
                    vvt[:, VW * j:VW * j + VW],
                    start=(cpv[s_] == 0),
                    stop=(cpv[s_] == npv[s_] - 1),
                )
                cpv[s_] += 1
                if cpv[s_] == npv[s_]:
                    src = acc[s_]
                    rec = rec_pool.tile([128, 1], fp32)
                    nc.vector.reciprocal(rec[:], src[:, 128:129])
                    ot = osb_pool.tile([128, 128], fp32)
                    nc.vector.tensor_scalar_mul(
                        ot[:], src[:, 0:128], rec[:])
                    # alternate the two HWDGE queues for output
                    eng = nc.sync if (i % 2 == 0) else nc.scalar
                    eng.dma_start(
                        out_d[h, b, 128 * i:128 * i + 128, :],
                        ot[:])
        if p == len(_PAIRS[g]) - 1:
            del gctx[(b, h, g)]

    emit_qk(0)
    for w in range(1, len(sched)):
        emit_qk(w)
        process(w - 1)
    process(len(sched) - 1)


def _mask_np():
    """[128, 256] bf16: cols 0:128 diag keep r<=c; cols 128:256 edge keep c<r."""
    r = np.arange(128)[:, None]
    c = np.arange(128)[None, :]
    diag = (r <= c).astype(np.float32)
    edge = (c < r).astype(np.float32)
    return np.concatenate([diag, edge], axis=1).astype(_BF16)


def _prep_in_maps(query, key, value):
    q = np.asarray(query, dtype=np.float32).reshape(B, S, H, D)
    k = np.asarray(key, dtype=np.float32).reshape(B, S, KVH, D)
    v = np.asarray(value, dtype=np.float32).reshape(B, S, KVH, D)

    # [H, D, B*S] / [KVH, D, B*S]
    qt_all = np.ascontiguousarray(q.transpose(2, 3, 0, 1).reshape(H, D, BS)).astype(_BF16)
    kt_all = np.ascontiguousarray(k.transpose(2, 3, 0, 1).reshape(KVH, D, BS)).astype(_BF16)

    # V with ones column, packed [KVH, B, 128p, NT*VW] so that
    # vv[c, b, p, t*VW + d] = V'[b, 128t + p, c, d]
    vpad = np.concatenate([v, np.ones((B, S, KVH, 1), np.float32)], axis=3)
    vv_all = np.ascontiguousarray(
        vpad.reshape(B, NT, 128, KVH, VW).transpose(3, 0, 2, 1, 4)
        .reshape(KVH, B, 128, NT * VW)).astype(_BF16)

    mk = _mask_np()
    return [
        {
            "qt": np.ascontiguousarray(qt_all[HQ * c:HQ * c + HQ]),
            "kt": np.ascontiguousarray(kt_all[c]),
            "vv": np.ascontiguousarray(vv_all[c]),
            "mk": mk,
        }
        for c in range(N_CORES)
    ]


def _assemble(results):
    # results[c]["out"]: [HQ, B, S, D] -> full [B, S, H*D]
    o = np.stack([np.asarray(results[c]["out"], dtype=np.float32)
                  for c in range(N_CORES)])  # [8, HQ, B, S, D]
    return np.ascontiguousarray(
        o.transpose(2, 3, 0, 1, 4).reshape(B, S, H * D))


def kernel(query, key, value):
    from concourse import bass_utils

    if "nc" not in _CACHE:
        _CACHE["nc"] = _build_nc()
    nc = _CACHE["nc"]
    in_maps = _prep_in_maps(query, key, value)
    res = bass_utils.run_bass_kernel_spmd(
        nc, in_maps, core_ids=list(range(N_CORES)))
    return _assemble(res.results)


# revision 10
# speedup vs baseline: 1.2060x; 1.2060x over previous
"""Trainium2 Bass kernel for GQA sliding-window causal attention.

Problem: B=2, S=2048, H=32 q-heads, KVH=8 kv-heads, D=128,
sliding window 1024, causal, scale 1/sqrt(128). f32 I/O.

Sharding (8 cores, pure tensor parallel, no collectives): core c gets
kv-head c and its query-head group [4c, 4c+4). Each core computes full
attention for its 4 q-heads over both batch elements; host concatenates
along the head dim.

Per-core algorithm (banded, no online softmax needed since scores are
O(1) and exp never overflows):
  - Q and K live in SBUF transposed: [d=128 partitions, s free].
  - Scores computed transposed, ST[k, q] = (KT_j).T-contracted-with-QT,
    per (512-wide q-block, 128-wide k-tile) over the causal+window band.
  - Band k-tiles are PAIRED into wide PSUM regions (a full 512-col tile
    in bank 0 + a ramp tile in bank 1, or two small ramp tiles packed in
    one bank) so ONE Exp activation covers up to 1024 contiguous columns
    -> 18 activations per (b,h) instead of 36, halving the fixed
    per-instruction overhead on the scalar engine (the critical
    resource).
  - P = exp(SCALE * ST) on ScalarE (scale folded into the activation),
    written as bf16 to SBUF.
  - Causal-diagonal and window-edge tiles are masked AFTER exp by
    multiplying with 0/1 bf16 mask tiles on VectorE (exact zeros).
  - PV: out[q, 0:129] += PT_slice.T @ V'_j where V' has a ones column
    appended -> col 128 accumulates the softmax denominator for free.
  - Normalize: out = psum[:, :128] * reciprocal(psum[:, 128]) on DVE.
  - Software pipelining: QK matmuls for pair p+1 are emitted before the
    exp/mask/PV processing of pair p, so the PE always has independent
    work while ScalarE runs (keeps the PE out of low p-states).
All matmuls bf16 with f32 PSUM accumulation; softmax math in f32.
"""

import numpy as np
import ml_dtypes

B = 2
S = 2048
H = 32
KVH = 8
D = 128
HQ = H // KVH  # q heads per core = 4
W = 1024  # sliding window
SCALE = 0.08838834764831845
N_CORES = 8
BS = B * S  # 4096
NT = S // 128  # 16 k-tiles / q-tiles per sequence
NG = S // 512  # 4 q-blocks per sequence
VW = D + 1  # 129: V width with ones column

_BF16 = ml_dtypes.bfloat16

_CACHE = {}


def _pairs_for_g(g):
    """Static schedule for q-block g: band tiles packed into ST pairs.

    Returns a list of (segs, width) where segs is a list of
    (j, qv, n, col_off) segments laid out contiguously (no garbage
    columns) in a [128, width] PSUM region; a full-512 first segment
    puts the second segment at col 512 (bank boundary).
    """
    q0 = 512 * g
    tiles = []
    for j in range(max(0, 4 * g - 8), 4 * g + 4):
        qv = max(q0, 128 * j)
        qe = min(q0 + 512, 128 * j + 128 + W)
        tiles.append((j, qv, qe - qv))
    fulls = [t for t in tiles if t[2] == 512]
    smalls = sorted([t for t in tiles if t[2] < 512], key=lambda t: -t[2])
    raw = []
    fi = si = 0
    while fi < len(fulls) and si < len(smalls):
        raw.append([fulls[fi], smalls[si]])
        fi += 1
        si += 1
    while fi + 1 < len(fulls):
        raw.append([fulls[fi], fulls[fi + 1]])
        fi += 2
    if fi < len(fulls):
        raw.append([fulls[fi]])
        fi += 1
    while si + 1 < len(smalls):
        a, b = smalls[si], smalls[si + 1]
        assert a[2] + b[2] <= 512
        raw.append([a, b])
        si += 2
    if si < len(smalls):
        raw.append([smalls[si]])
        si += 1
    pairs = []
    for p in raw:
        segs = []
        off = 0
        for (j, qv, n) in p:
            segs.append((j, qv, n, off))
            off = 512 if (off == 0 and n == 512) else off + n
        width = segs[-1][3] + segs[-1][2]
        pairs.append((segs, width))
    return pairs


_PAIRS = [_pairs_for_g(g) for g in range(NG)]


def _build_nc(reps=1, loop_reps=0, opts=None):
    """Build + compile the single-core Bass/Tile program (SPMD across 8).

    reps > 1 unrolls the whole computation inside one NEFF; loop_reps > 0
    instead wraps the body in a hardware For_i loop. Both are used only
    for timing. opts: dict of tuning switches (see _body_once).
    """
    from contextlib import ExitStack

    import concourse.bass as bass
    import concourse.tile as tile
    from concourse import bacc, mybir

    opts = dict(opts or {})
    fp32 = mybir.dt.float32
    bf16 = mybir.dt.bfloat16

    nc = bacc.Bacc("TRN2", target_bir_lowering=False, debug=False,
                   num_devices=N_CORES)

    qt_d = nc.dram_tensor("qt", [HQ, D, BS], bf16, kind="ExternalInput").ap()
    kt_d = nc.dram_tensor("kt", [D, BS], bf16, kind="ExternalInput").ap()
    vv_d = nc.dram_tensor("vv", [B, 128, NT * VW], bf16, kind="ExternalInput").ap()
    mk_d = nc.dram_tensor("mk", [128, 256], bf16, kind="ExternalInput").ap()
    out_d = nc.dram_tensor("out", [HQ, B, S, D], fp32, kind="ExternalOutput").ap()

    with tile.TileContext(nc) as tc, ExitStack() as ctx:
        mask_pool = ctx.enter_context(tc.tile_pool(name="mask", bufs=1))
        kt_pool = ctx.enter_context(tc.tile_pool(name="ktp", bufs=2))
        vv_pool = ctx.enter_context(tc.tile_pool(name="vvp", bufs=2))
        qt_pool = ctx.enter_context(tc.tile_pool(name="qtp", bufs=2))
        pt_pool = ctx.enter_context(tc.tile_pool(name="ptp",
                                                 bufs=opts.get("pt_bufs", 6)))
        osb_pool = ctx.enter_context(tc.tile_pool(name="osb", bufs=6))
        rec_pool = ctx.enter_context(tc.tile_pool(name="rec", bufs=6))
        st_pool = ctx.enter_context(
            tc.tile_pool(name="stp", bufs=opts.get("st_bufs", 2), space="PSUM"))
        acc_pool = ctx.enter_context(
            tc.tile_pool(name="accp", bufs=opts.get("acc_bufs", 4),
                         space="PSUM"))

        masks = mask_pool.tile([128, 256], bf16)
        nc.sync.dma_start(masks[:], mk_d[:])

        pools = (kt_pool, vv_pool, qt_pool, pt_pool, osb_pool, rec_pool,
                 st_pool, acc_pool)
        if loop_reps:
            with tc.For_i(0, loop_reps, 1,
                          hint_engines=tuple(nc.engines)) as _i:
                _body_once(nc, tc, mybir, masks, *pools,
                           qt_d, kt_d, vv_d, out_d, opts)
        else:
            for _rep in range(reps):
                _body_once(nc, tc, mybir, masks, *pools,
                           qt_d, kt_d, vv_d, out_d, opts)

    nc.compile()
    return nc


def _body_once(nc, tc, mybir, masks, kt_pool, vv_pool, qt_pool, pt_pool,
               osb_pool, rec_pool, st_pool, acc_pool, qt_d, kt_d, vv_d,
               out_d, opts=None):
    opts = opts or {}
    split_act = opts.get("split_act", False)  # one ACT per segment (fallback)
    fp32 = mybir.dt.float32
    bf16 = mybir.dt.bfloat16

    # prefetched input tiles, keyed (kind, b[, h]). All input DMAs are
    # chunked (~128 KB) and round-robined over the two HWDGE queues so the
    # first QK can start ~1.5 us in and later prefetches never monopolize
    # a queue. Chunks are 512-col (q/k) / 4-ktile (v) aligned so every
    # compute read lands inside a single chunk (clean subtile deps).
    kv_tiles = {}
    _qrr = [0]

    def _dma_chunk(dst, src, c0, c1):
        eng = nc.sync if _qrr[0] % 2 == 0 else nc.scalar
        _qrr[0] += 1
        eng.dma_start(dst[:, c0:c1], src[:, c0:c1])

    def load_kv(b, interleave_q=None):
        if ("k", b) in kv_tiles:
            return
        ktt = kt_pool.tile([128, S], bf16, name=f"ktt_{b}")
        vvt = vv_pool.tile([128, NT * VW], bf16, name=f"vvt_{b}")
        kv_tiles[("k", b)] = ktt
        kv_tiles[("v", b)] = vvt
        ksrc = kt_d[:, b * S:(b + 1) * S]
        qtt = qsrc = None
        if interleave_q is not None:
            h = interleave_q
            qtt = qt_pool.tile([128, S], bf16, name=f"qtt_{b}_{h}")
            kv_tiles[("q", b, h)] = qtt
            qsrc = qt_d[h, :, b * S:(b + 1) * S]
        # first-needed chunks first, strictly alternating the two queues
        for c in range(2):
            _dma_chunk(ktt, ksrc, 1024 * c, 1024 * (c + 1))
            if qtt is not None:
                _dma_chunk(qtt, qsrc, 1024 * c, 1024 * (c + 1))
            _dma_chunk(vvt, vv_d[b], 8 * VW * c, 8 * VW * (c + 1))

    def load_q(b, h):
        if ("q", b, h) not in kv_tiles:
            qtt = qt_pool.tile([128, S], bf16, name=f"qtt_{b}_{h}")
            for c in range(2):
                _dma_chunk(qtt, qt_d[h, :, b * S:(b + 1) * S],
                           1024 * c, 1024 * (c + 1))
            kv_tiles[("q", b, h)] = qtt

    load_kv(0, interleave_q=0)

    # flat schedule of all score-pairs across (b, h, g) so the software
    # pipeline (QK of pair w+1 ahead of processing of pair w) crosses
    # q-block/head/batch boundaries without draining
    sched = []
    for b in range(B):
        for h in range(HQ):
            for g in range(NG):
                for p in range(len(_PAIRS[g])):
                    sched.append((b, h, g, p))

    gctx = {}  # (b, h, g) -> {'acc': [...], 'cpv': [...], 'st': [...]}

    def emit_qk(w):
        b, h, g, p = sched[w]
        pairs = _PAIRS[g]
        if p == 0:
            gctx[(b, h, g)] = {
                "cpv": [0, 0, 0, 0],
                "st": [None] * len(pairs),
            }
            if g == 0:
                # prefetch next head's Q (or next batch's K/V/Q)
                if h + 1 < HQ:
                    load_q(b, h + 1)
                elif b + 1 < B:
                    load_kv(b + 1)
                    load_q(b + 1, 0)
        ktt = kv_tiles[("k", b)]
        qtt = kv_tiles[("q", b, h)]
        segs, width = pairs[p]
        st = st_pool.tile([128, 1024], fp32, tag="st",
                          name=f"st_{b}_{h}_{g}_{p}")
        for (j, qv, n, off) in segs:
            nc.tensor.matmul(
                st[:, off:off + n],
                ktt[:, 128 * j:128 * j + 128],
                qtt[:, qv:qv + n],
                start=True, stop=True,
            )
        gctx[(b, h, g)]["st"][p] = st

    def front(w):
        # exp + masks for pair w (runs one pair ahead of back(w) so mask
        # ops enter the DVE queue before the previous pair's finalizers)
        b, h, g, p = sched[w]
        segs, width = _PAIRS[g][p]
        ctx = gctx[(b, h, g)]
        st = ctx["st"][p]
        pt = pt_pool.tile([128, 1024], bf16, tag="pt",
                          name=f"pt_{b}_{h}_{g}_{p}")
        ctx["pt"] = ctx.get("pt", {})
        ctx["pt"][p] = pt
        if split_act:
            for (j, qv, n, off) in segs:
                nc.scalar.activation(
                    pt[:, off:off + n], st[:, off:off + n],
                    mybir.ActivationFunctionType.Exp, scale=SCALE)
        else:
            nc.scalar.activation(
                pt[:, 0:width], st[:, 0:width],
                mybir.ActivationFunctionType.Exp, scale=SCALE)
        for (j, qv, n, off) in segs:
            if j >= 4 * g:
                # causal diagonal tile: first 128 cols of seg
                nc.vector.tensor_mul(
                    pt[:, off:off + 128], pt[:, off:off + 128],
                    masks[:, 0:128])
            if qv + n == 128 * j + 128 + W:
                # window edge tile: last 128 cols of seg
                nc.vector.tensor_mul(
                    pt[:, off + n - 128:off + n],
                    pt[:, off + n - 128:off + n],
                    masks[:, 128:256])

    def back(w):
        # PV accumulation + finalizers for pair w
        b, h, g, p = sched[w]
        segs, width = _PAIRS[g][p]
        ctx = gctx[(b, h, g)]
        if "acc" not in ctx:
            ctx["acc"] = [acc_pool.tile([128, VW], fp32, tag="acc",
                                        name=f"acc_{b}_{h}_{g}_{s_}")
                          for s_ in range(4)]
        acc = ctx["acc"]
        cpv = ctx["cpv"]
        npv = [min(4 * g + s_, 8) + 1 for s_ in range(4)]
        vvt = kv_tiles[("v", b)]
        pt = ctx["pt"][p]
        for (j, qv, n, off) in segs:
            for i in range(max(4 * g, j), min(4 * g + 3, j + 8) + 1):
                s_ = i - 4 * g
                po = off + 128 * i - qv
                nc.tensor.matmul(
                    acc[s_][:, :],
                    pt[:, po:po + 128],
                    vvt[:, VW * j:VW * j + VW],
                    start=(cpv[s_] == 0),
                    stop=(cpv[s_] == npv[s_] - 1),
                )
                cpv[s_] += 1
                if cpv[s_] == npv[s_]:
                    src = acc[s_]
                    rec = rec_pool.tile([128, 1], fp32)
                    nc.vector.reciprocal(rec[:], src[:, 128:129])
                    ot = osb_pool.tile([128, 128], fp32)
                    nc.vector.tensor_scalar_mul(
                        ot[:], src[:, 0:128], rec[:])
                    nc.sync.dma_start(
                        out_d[h, b, 128 * i:128 * i + 128, :],
                        ot[:])
        if p == len(_PAIRS[g]) - 1:
            del gctx[(b, h, g)]

    # 3-stage software pipeline: QK(w) | exp+mask(w-1) | PV+finalize(w-2)
    nsched = len(sched)
    for w in range(nsched + 2):
        if w < nsched:
            emit_qk(w)
        if 1 <= w < nsched + 1:
            front(w - 1)
        if w >= 2:
            back(w - 2)


def _mask_np():
    """[128, 256] bf16: cols 0:128 diag keep r<=c; cols 128:256 edge keep c<r."""
    r = np.arange(128)[:, None]
    c = np.arange(128)[None, :]
    diag = (r <= c).astype(np.float32)
    edge = (c < r).astype(np.float32)
    return np.concatenate([diag, edge], axis=1).astype(_BF16)


def _prep_in_maps(query, key, value):
    q = np.asarray(query, dtype=np.float32).reshape(B, S, H, D)
    k = np.asarray(key, dtype=np.float32).reshape(B, S, KVH, D)
    v = np.asarray(value, dtype=np.float32).reshape(B, S, KVH, D)

    # [H, D, B*S] / [KVH, D, B*S]
    qt_all = np.ascontiguousarray(q.transpose(2, 3, 0, 1).reshape(H, D, BS)).astype(_BF16)
    kt_all = np.ascontiguousarray(k.transpose(2, 3, 0, 1).reshape(KVH, D, BS)).astype(_BF16)

    # V with ones column, packed [KVH, B, 128p, NT*VW] so that
    # vv[c, b, p, t*VW + d] = V'[b, 128t + p, c, d]
    vpad = np.concatenate([v, np.ones((B, S, KVH, 1), np.float32)], axis=3)
    vv_all = np.ascontiguousarray(
        vpad.reshape(B, NT, 128, KVH, VW).transpose(3, 0, 2, 1, 4)
        .reshape(KVH, B, 128, NT * VW)).astype(_BF16)

    mk = _mask_np()
    return [
        {
            "qt": np.ascontiguousarray(qt_all[HQ * c:HQ * c + HQ]),
            "kt": np.ascontiguousarray(kt_all[c]),
            "vv": np.ascontiguousarray(vv_all[c]),
            "mk": mk,
        }
        for c in range(N_CORES)
    ]


def _assemble(results):
    # results[c]["out"]: [HQ, B, S, D] -> full [B, S, H*D]
    o = np.stack([np.asarray(results[c]["out"], dtype=np.float32)
                  for c in range(N_CORES)])  # [8, HQ, B, S, D]
    return np.ascontiguousarray(
        o.transpose(2, 3, 0, 1, 4).reshape(B, S, H * D))


def kernel(query, key, value):
    from concourse import bass_utils

    if "nc" not in _CACHE:
        _CACHE["nc"] = _build_nc()
    nc = _CACHE["nc"]
    in_maps = _prep_in_maps(query, key, value)
    res = bass_utils.run_bass_kernel_spmd(
        nc, in_maps, core_ids=list(range(N_CORES)))
    return _assemble(res.results)
